# revision 20
# baseline (speedup 1.0000x reference)
"""Trainium2 Bass kernel for the multimodal GRU-D-style LSTM imputation model.

Self-contained: kernel(**inputs) takes the FULL inputs (B=4096) and returns
(loss, sigmoid(y_h), imputations) exactly like the reference.

Strategy: pure data-parallel over 8 NeuronCores (B=512 per core). Each core
runs the full T=128-step scan on its batch shard with bf16 matmuls
(fp32 PSUM accumulation), one activation-table set (exp/tanh/relu) by
expressing sigmoid via tanh, and fp32 state for the LSTM cell c.
Scalar losses are finished on the host from per-core partial sums.

Layout notes (per core, B=512 -> 4 partition chunks of 128):
  - transposed activations live as [feat, batch] tiles: h~=2h bf16 [128,2048]
    fat tiles (col block k = H rows 128k..128k+127), c~=2c fp32 [128,2048].
  - per-step inputs x/m/d loaded natural [128(b), k, t, 96], converted to
    bf16 (f padded to 128 with zeros) and transposed via the DMA xbar.
  - imputations c_c computed in natural layout fp32 (exact where mask=1) and
    DMA'd straight out; the transposed bf16 copy feeds the LSTM gates.
  - sigmoid(z) = (1+tanh(z/2))/2 folded into pre-scaled weights: the kernel
    carries h~=2h and c~=2c; hist_W/Whh/out_W are pre-halved, d3 pre-doubled.
"""

import numpy as np
import ml_dtypes

import concourse.bass as bass
import concourse.bacc as bacc
import concourse.tile as tile
from concourse import mybir

F32 = mybir.dt.float32
BF16 = mybir.dt.bfloat16
FP8 = mybir.dt.float8e4
USE_FP8_HH = True
FP8_K = 16.0
AF = mybir.ActivationFunctionType
ALU = mybir.AluOpType
AX = mybir.AxisListType

B_FULL, T_FULL, FD, S, H = 4096, 128, 96, 128, 512
N_CORES = 8
IMPUTE_WEIGHT, L1, L2 = 0.3, 0.01, 0.1


def build_module(Bc=512, T=128, Tc=4, repeat=1):
    """Build the per-core Bass module. Bc = per-core batch, T = steps."""
    NB = Bc // 128          # batch chunks
    NH = H // 128           # hidden chunks (4)
    NG = 4 * H // 128       # gate M-chunks (16)
    assert Bc % 128 == 0 and T % Tc == 0

    nc = bacc.Bacc(target_bir_lowering=False)

    # ---- DRAM I/O ----
    vals = nc.declare_dram_parameter("values", [Bc, T, FD], F32, isOutput=False)
    msks = nc.declare_dram_parameter("masks", [Bc, T, FD], F32, isOutput=False)
    dels = nc.declare_dram_parameter("deltas", [Bc, T, FD], F32, isOutput=False)
    stat = nc.declare_dram_parameter("statics", [Bc, S], F32, isOutput=False)
    smk = nc.declare_dram_parameter("smasks", [Bc, S], F32, isOutput=False)

    w_fr = nc.declare_dram_parameter("w_fr", [S, S], BF16, isOutput=False)        # fr_W.T
    w_d1 = nc.declare_dram_parameter("w_d1", [S, H], BF16, isOutput=False)        # d1_W.T
    w_d2 = nc.declare_dram_parameter("w_d2", [H, H], BF16, isOutput=False)        # d2_W.T
    w_d3 = nc.declare_dram_parameter("w_d3", [H, H], BF16, isOutput=False)        # (2*d3_W).T
    w_td = nc.declare_dram_parameter("w_td", [FD + 1, H], BF16, isOutput=False)   # [td_W.T; td_b]
    w_hist = nc.declare_dram_parameter("w_hist", [H, FD], BF16, isOutput=False)   # (hist_W/2).T
    w_st = nc.declare_dram_parameter("w_st", [S, FD], BF16, isOutput=False)       # st_W.T
    w_real = nc.declare_dram_parameter("w_real", [FD, FD], BF16, isOutput=False)  # real_W.T
    w_icc = nc.declare_dram_parameter("w_icc", [FD, 4 * H], BF16, isOutput=False)  # Wih[:, :96].T
    w_im = nc.declare_dram_parameter("w_im", [FD + 1, 4 * H], BF16, isOutput=False)  # [Wih_m.T; b], ifo cols halved
    if USE_FP8_HH:
        w_hh = nc.declare_dram_parameter("w_hh", [2, 128, 2, 4 * H], FP8, isOutput=False)
    else:
        w_hh = nc.declare_dram_parameter("w_hh", [H, 4 * H], BF16, isOutput=False)  # (Whh/2).T
    w_out = nc.declare_dram_parameter("w_out", [H, 1], BF16, isOutput=False)       # (out_W/2).T

    b_fr = nc.declare_dram_parameter("b_fr", [S, 1], F32, isOutput=False)
    b_d1 = nc.declare_dram_parameter("b_d1", [128, NH], F32, isOutput=False)
    b_d2 = nc.declare_dram_parameter("b_d2", [128, NH], F32, isOutput=False)
    b_d3 = nc.declare_dram_parameter("b_d3", [128, NH], F32, isOutput=False)      # 2*d3_b
    b_stc = nc.declare_dram_parameter("b_stc", [FD, 1], F32, isOutput=False)      # st_b+real_b+hist_b
    ident = nc.declare_dram_parameter("ident", [FD, FD], BF16, isOutput=False)

    o_imps = nc.declare_dram_parameter("imps", [Bc, T, FD], F32, isOutput=True)
    o_y = nc.declare_dram_parameter("ylog", [1, Bc], F32, isOutput=True)
    o_xnum = nc.declare_dram_parameter("xnum", [128, T], F32, isOutput=True)
    o_xden = nc.declare_dram_parameter("xden", [128, T], F32, isOutput=True)

    vals_v = vals.ap().rearrange("(k p) t f -> p k t f", k=NB)
    msks_v = msks.ap().rearrange("(k p) t f -> p k t f", k=NB)
    dels_v = dels.ap().rearrange("(k p) t f -> p k t f", k=NB)
    imps_v = o_imps.ap().rearrange("(k p) t f -> p k t f", k=NB)
    stat_v = stat.ap().rearrange("(k p) s -> p k s", k=NB)
    smk_v = smk.ap().rearrange("(k p) s -> p k s", k=NB)

    FB = NH * Bc  # fat tile free size (2048 at Bc=512)

    with tile.TileContext(nc) as tc:
        with tc.tile_pool(name="wp", bufs=1) as wp, \
             tc.tile_pool(name="sp1", bufs=1) as sp1, \
             tc.tile_pool(name="sp2", bufs=2) as sp2, \
             tc.tile_pool(name="psG", bufs=3, space="PSUM") as psG, \
             tc.tile_pool(name="psB", bufs=2, space="PSUM") as psB:

            # ---- persistent loop weights ----
            t_td = wp.tile([FD + 1, H], BF16, name="t_td")
            nc.sync.dma_start(out=t_td, in_=w_td[:, :])
            t_hist = [wp.tile([128, FD], BF16, tag=f"hi_{k}", name=f"hi_{k}") for k in range(NH)]
            for k in range(NH):
                nc.sync.dma_start(out=t_hist[k], in_=w_hist[128 * k:128 * (k + 1), :])
            t_real = wp.tile([FD, FD], BF16, name="t_real")
            nc.sync.dma_start(out=t_real, in_=w_real[:, :])
            t_icc = wp.tile([FD, 4 * H], BF16, name="t_icc")
            nc.sync.dma_start(out=t_icc, in_=w_icc[:, :])
            t_im = wp.tile([FD + 1, 4 * H], BF16, name="t_im")
            nc.sync.dma_start(out=t_im, in_=w_im[:, :])
            if USE_FP8_HH:
                t_hh = [wp.tile([128, 2, 4 * H], FP8, tag=f"hh_{k}", name=f"hh_{k}")
                        for k in range(2)]
                for k in range(2):
                    nc.sync.dma_start(out=t_hh[k], in_=w_hh[k, :, :, :])
            else:
                t_hh = [wp.tile([128, 4 * H], BF16, tag=f"hh_{k}", name=f"hh_{k}")
                        for k in range(NH)]
                for k in range(NH):
                    nc.sync.dma_start(out=t_hh[k], in_=w_hh[128 * k:128 * (k + 1), :])
            t_out = [wp.tile([128, 1], BF16, tag=f"ow_{k}", name=f"ow_{k}") for k in range(NH)]
            for k in range(NH):
                nc.sync.dma_start(out=t_out[k], in_=w_out[128 * k:128 * (k + 1), :])
            t_stcb = wp.tile([FD, 1], F32, name="t_stcb")
            nc.sync.dma_start(out=t_stcb, in_=b_stc[:, :])
            t_stc = wp.tile([FD, Bc], F32, name="t_stc")
            t_idn = wp.tile([FD, FD], BF16, name="t_idn")
            nc.sync.dma_start(out=t_idn, in_=ident[:, :])

            # persistent accumulators / padded bf16 window tiles
            t_xnum = wp.tile([128, T], F32, name="t_xnum")
            t_xden = wp.tile([128, T], F32, name="t_xden")
            vb = [wp.tile([128, NB, Tc, 128], BF16, tag=f"vb{i}", name=f"vb{i}") for i in range(2)]
            mb = [wp.tile([128, NB, Tc, 128], BF16, tag=f"mb{i}", name=f"mb{i}") for i in range(2)]
            db = [wp.tile([128, NB, Tc, 128], BF16, tag=f"db{i}", name=f"db{i}") for i in range(2)]
            for tl_ in vb + mb + db:
                nc.gpsimd.memset(tl_, 0.0)
            for tl_ in mb + db:
                nc.gpsimd.memset(tl_[:, :, :, FD:FD + 1], 1.0)

            # ---- prologue: statics -> s_cT, h0, st_term (pool freed after) ----
            with tc.tile_pool(name="pre", bufs=1) as pre:
                t_fr = pre.tile([S, S], BF16, name="t_fr")
                nc.sync.dma_start(out=t_fr, in_=w_fr[:, :])
                t_d1 = pre.tile([S, H], BF16, name="t_d1")
                nc.sync.dma_start(out=t_d1, in_=w_d1[:, :])
                t_d2 = [pre.tile([128, H], BF16, tag=f"d2_{k}", name=f"d2_{k}") for k in range(NH)]
                t_d3 = [pre.tile([128, H], BF16, tag=f"d3_{k}", name=f"d3_{k}") for k in range(NH)]
                for k in range(NH):
                    nc.sync.dma_start(out=t_d2[k], in_=w_d2[128 * k:128 * (k + 1), :])
                    nc.sync.dma_start(out=t_d3[k], in_=w_d3[128 * k:128 * (k + 1), :])
                t_frb = pre.tile([S, 1], F32, name="t_frb")
                nc.sync.dma_start(out=t_frb, in_=b_fr[:, :])
                t_d1b = pre.tile([128, NH], F32, name="t_d1b")
                nc.sync.dma_start(out=t_d1b, in_=b_d1[:, :])
                t_d2b = pre.tile([128, NH], F32, name="t_d2b")
                nc.sync.dma_start(out=t_d2b, in_=b_d2[:, :])
                t_d3b = pre.tile([128, NH], F32, name="t_d3b")
                nc.sync.dma_start(out=t_d3b, in_=b_d3[:, :])

                st_nat = pre.tile([128, NB, S], F32, name="st_nat")
                nc.sync.dma_start(out=st_nat, in_=stat_v[:, :, :])
                sm_nat = pre.tile([128, NB, S], F32, name="sm_nat")
                nc.sync.dma_start(out=sm_nat, in_=smk_v[:, :, :])
                st_b16 = pre.tile([128, NB, S], BF16, name="st_b16")
                nc.gpsimd.tensor_copy(st_b16, st_nat)
                sm_b16 = pre.tile([128, NB, S], BF16, name="sm_b16")
                nc.gpsimd.tensor_copy(sm_b16, sm_nat)
                stT = pre.tile([S, Bc], BF16, name="stT")
                smT = pre.tile([S, Bc], BF16, name="smT")
                for k in range(NB):
                    nc.sync.dma_start_transpose(stT[:, 128 * k:128 * (k + 1)], st_b16[:, k, :])
                    nc.sync.dma_start_transpose(smT[:, 128 * k:128 * (k + 1)], sm_b16[:, k, :])
                sh_ps = psG.tile([S, Bc], F32, tag="g", name="sh_ps")
                nc.tensor.matmul(sh_ps, t_fr, stT, start=True, stop=True)
                sh = pre.tile([S, Bc], BF16, name="sh")
                nc.scalar.activation(out=sh, in_=sh_ps, func=AF.Identity, bias=t_frb, scale=1.0)
                sdif = pre.tile([S, Bc], BF16, name="sdif")
                nc.vector.tensor_sub(sdif, stT, sh)
                nc.vector.tensor_mul(sdif, smT, sdif)
                scT = pre.tile([S, Bc], BF16, name="scT")
                nc.vector.tensor_add(scT, sh, sdif)

                # dense chain h1 -> h2 -> h3 (h3 doubled via doubled d3 weights)
                h1 = pre.tile([128, FB], BF16, name="h1")
                for j in range(NH):
                    ps = psG.tile([128, Bc], F32, tag="g", name=f"h1ps{j}")
                    nc.tensor.matmul(ps, t_d1[:, 128 * j:128 * (j + 1)], scT,
                                     start=True, stop=True)
                    nc.scalar.activation(out=h1[:, Bc * j:Bc * (j + 1)], in_=ps, func=AF.Relu,
                                         bias=t_d1b[:, j:j + 1], scale=1.0)
                h2 = pre.tile([128, FB], BF16, name="h2")
                for j in range(NH):
                    ps = psG.tile([128, Bc], F32, tag="g", name=f"h2ps{j}")
                    for k in range(NH):
                        nc.tensor.matmul(ps, t_d2[k][:, 128 * j:128 * (j + 1)],
                                         h1[:, Bc * k:Bc * (k + 1)],
                                         start=(k == 0), stop=(k == NH - 1))
                    nc.scalar.activation(out=h2[:, Bc * j:Bc * (j + 1)], in_=ps, func=AF.Relu,
                                         bias=t_d2b[:, j:j + 1], scale=1.0)
                h_cur = sp2.tile([128, FB], BF16, tag="h", name="h0")
                for j in range(NH):
                    ps = psG.tile([128, Bc], F32, tag="g", name=f"h3ps{j}")
                    for k in range(NH):
                        nc.tensor.matmul(ps, t_d3[k][:, 128 * j:128 * (j + 1)],
                                         h2[:, Bc * k:Bc * (k + 1)],
                                         start=(k == 0), stop=(k == NH - 1))
                    nc.scalar.activation(out=h_cur[:, Bc * j:Bc * (j + 1)], in_=ps, func=AF.Relu,
                                         bias=t_d3b[:, j:j + 1], scale=1.0)

                stc_ps = psB.tile([FD, Bc], F32, tag="xc", name="stc_ps")
                nc.tensor.matmul(stc_ps, t_st_pre(nc, pre, w_st), scT, start=True, stop=True)
                nc.scalar.activation(out=t_stc, in_=stc_ps, func=AF.Identity,
                                     bias=t_stcb, scale=1.0)

            c_cur = sp2.tile([128, FB], BF16, tag="c", name="c0")
            nc.vector.memset(c_cur, 0.0)

            with tc.tile_pool(name="win", bufs=2) as win, \
                 tc.tile_pool(name="tp", bufs=2) as tp, \
                 tc.tile_pool(name="xp", bufs=2) as xp, \
                 tc.tile_pool(name="np_", bufs=2) as npo:

                wins = {}

                def load_window(w, lbl=None):
                    lbl = w if lbl is None else f"{w}_{lbl}"
                    t0 = w * Tc
                    wpi = w % 2
                    vf = win.tile([128, NB, Tc, FD], F32, tag="vf", name=f"vf{lbl}")
                    nc.sync.dma_start(out=vf, in_=vals_v[:, :, t0:t0 + Tc, :])
                    mf = win.tile([128, NB, Tc, FD], F32, tag="mf", name=f"mf{lbl}")
                    nc.sync.dma_start(out=mf, in_=msks_v[:, :, t0:t0 + Tc, :])
                    df = win.tile([128, NB, Tc, FD], F32, tag="df", name=f"df{lbl}")
                    nc.sync.dma_start(out=df, in_=dels_v[:, :, t0:t0 + Tc, :])
                    nc.gpsimd.tensor_copy(vb[wpi][:, :, :, 0:FD], vf)
                    nc.gpsimd.tensor_copy(mb[wpi][:, :, :, 0:FD], mf)
                    nc.gpsimd.tensor_copy(db[wpi][:, :, :, 0:FD], df)
                    nc.vector.tensor_reduce(
                        out=t_xden[:, t0:t0 + Tc], in_=mf.transpose([0, 2, 1, 3]),
                        axis=AX.XY, op=ALU.add)
                    wins[w] = (vf, mf, df)

                def transposes(t, lbl=None):
                    lbl = t if lbl is None else f"{t}_{lbl}"
                    w, tl = t // Tc, t % Tc
                    wpi = w % 2
                    vT = tp.tile([128, Bc], BF16, tag="vT", name=f"vT{lbl}")
                    mT = tp.tile([128, Bc], BF16, tag="mT", name=f"mT{lbl}")
                    dT = tp.tile([128, Bc], BF16, tag="dT", name=f"dT{lbl}")
                    for k in range(NB):
                        nc.sync.dma_start_transpose(vT[:, 128 * k:128 * (k + 1)],
                                                    vb[wpi][:, k, tl, :])
                        nc.sync.dma_start_transpose(mT[:, 128 * k:128 * (k + 1)],
                                                    mb[wpi][:, k, tl, :])
                        nc.sync.dma_start_transpose(dT[:, 128 * k:128 * (k + 1)],
                                                    db[wpi][:, k, tl, :])
                    return vT, mT, dT

                def gamma_of(t, dT):
                    # gamma = min(exp(-(z + td_b)), 1): bias via ones-row K=97
                    gam = sp2.tile([128, FB], BF16, tag="gam", name=f"gam{t}")
                    for j2 in range(NH // 2):
                        ps = psG.tile([128, 2 * Bc], F32, tag="g", name=f"gps{t}_{j2}")
                        for q in range(2):
                            j = 2 * j2 + q
                            nc.tensor.matmul(ps[:, Bc * q:Bc * (q + 1)],
                                             t_td[:, 128 * j:128 * (j + 1)],
                                             dT[0:FD + 1, :], start=True, stop=True)
                        nc.scalar.activation(out=gam[:, 2 * Bc * j2:2 * Bc * (j2 + 1)],
                                             in_=ps, func=AF.Exp, scale=-1.0)
                    nc.vector.tensor_scalar_min(gam, gam, 1.0)
                    return gam

                def make_hp8(t):
                    return sp2.tile([128, 2, 2, Bc], FP8, tag="hp8", name=f"hp8{t}")

                # ---- startup: window 0, transposes/gamma/hp for t=0 ----
                load_window(0)
                cur = hp = hp8 = None
                for nt in range(repeat * T):
                    t = nt % T
                    if t == 0:  # (re)start a pass (repeat>1 is a timing rig)
                        cur = transposes(0) if nt == 0 else transposes(0, nt)
                        hp = gamma_of(nt, cur[2])
                        nc.vector.tensor_mul(hp, h_cur, hp)
                        hp8 = None
                        if USE_FP8_HH:
                            hp8 = make_hp8(nt)
                            nc.gpsimd.tensor_copy(
                                hp8, hp[:, :].rearrange("p (b e n) -> p b e n", b=2, e=2))
                    w, tl = t // Tc, t % Tc
                    if tl == 0 and w + 1 < T // Tc:
                        load_window(w + 1, nt)
                    vf, mf, df = wins[w]
                    vT, mT, dT = cur
                    if t + 1 < T:
                        nxt = transposes(t + 1, nt + 1)
                        gam_next = gamma_of(nt + 1, nxt[2])
                        hp8_next = make_hp8(nt + 1) if USE_FP8_HH else None
                    else:
                        nxt = gam_next = hp8_next = None

                    # x_cT = real_W @ xT + (hist_W/2) @ hp + st_term(+biases)
                    xps = psB.tile([FD, Bc], F32, tag="xc", name=f"xc{nt}")
                    nc.tensor.matmul(xps, t_real, vT[0:FD, :], start=True, stop=False)
                    for k in range(NH):
                        nc.tensor.matmul(xps, t_hist[k], hp[:, Bc * k:Bc * (k + 1)],
                                         start=False, stop=(k == NH - 1))
                    xcb = xp.tile([FD, Bc], BF16, tag="xcb", name=f"xcb{nt}")
                    nc.vector.tensor_add(xcb, xps, t_stc)

                    # transposed blend for the gate input c_cT
                    difT = xp.tile([FD, Bc], BF16, tag="difT", name=f"difT{nt}")
                    nc.vector.tensor_sub(difT, vT[0:FD, :], xcb)
                    nc.vector.tensor_mul(difT, mT[0:FD, :], difT)
                    ccT = xp.tile([FD, Bc], BF16, tag="ccT", name=f"ccT{nt}")
                    nc.vector.tensor_add(ccT, xcb, difT)

                    # LSTM gates, biases+0.5-scales folded into the GEMM:
                    # per 128-row chunk the blocks (g,i)/(f,o) share one
                    # 2-bank psum + one ACT; the cell/hidden tail and the
                    # NEXT step's decayed hidden pipeline per chunk.
                    c_new = sp2.tile([128, FB], BF16, tag="c", name=f"c{nt}")
                    tau_c = sp1.tile([128, FB], BF16, tag="tc", name=f"tc{nt}")
                    h_new = sp2.tile([128, FB], BF16, tag="h", name=f"h{nt}")
                    for col in range(NH):
                        ga = sp2.tile([128, 4, Bc], BF16, tag="ga", name=f"ga{nt}_{col}")
                        for pair in ((2, 0), (1, 3)):  # (g,i) then (f,o)
                            ps = psG.tile([128, 2 * Bc], F32, tag="g",
                                          name=f"g{nt}_{col}_{pair[0]}")
                            for q, gate in enumerate(pair):
                                j = gate * NH + col
                                pso = ps[:, Bc * q:Bc * (q + 1)]
                                nc.tensor.matmul(pso, t_im[:, 128 * j:128 * (j + 1)],
                                                 mT[0:FD + 1, :], start=True, stop=False)
                                if USE_FP8_HH:
                                    for k in range(2):
                                        nc.tensor.matmul(
                                            pso, t_hh[k][:, :, 128 * j:128 * (j + 1)],
                                            hp8[:, k, :, :], start=False, stop=False,
                                            perf_mode=mybir.MatmulPerfMode.DoubleRow)
                                else:
                                    for k in range(NH):
                                        nc.tensor.matmul(
                                            pso, t_hh[k][:, 128 * j:128 * (j + 1)],
                                            hp[:, Bc * k:Bc * (k + 1)],
                                            start=False, stop=False)
                                nc.tensor.matmul(pso, t_icc[:, 128 * j:128 * (j + 1)], ccT,
                                                 start=False, stop=True)
                            blk = 0 if pair[0] == 2 else 2
                            nc.scalar.activation(out=ga[:, blk:blk + 2, :], in_=ps,
                                                 func=AF.Tanh,
                                                 scale=(1.0 / FP8_K) if USE_FP8_HH else 1.0)
                        g_b, i_b, f_b, o_b = ga[:, 0, :], ga[:, 1, :], ga[:, 2, :], ga[:, 3, :]
                        cs = slice(Bc * col, Bc * (col + 1))
                        # c~_new = (0.5+0.5*tau_f)*c~ + (1+tau_i)*g   (chunk col)
                        nc.vector.tensor_scalar(f_b, f_b, 0.5, 0.5, ALU.mult, ALU.add)
                        nc.vector.tensor_scalar(i_b, i_b, 1.0, None, ALU.add)
                        nc.vector.tensor_mul(g_b, i_b, g_b)
                        nc.vector.tensor_mul(c_cur[:, cs], f_b, c_cur[:, cs])
                        nc.vector.tensor_add(c_new[:, cs], c_cur[:, cs], g_b)
                        # h~_new = (1+tau_o)*tau_c,  tau_c = tanh(c~/2)
                        nc.scalar.activation(out=tau_c[:, cs], in_=c_new[:, cs],
                                             func=AF.Tanh, scale=0.5)
                        nc.vector.tensor_scalar(o_b, o_b, 1.0, None, ALU.add)
                        nc.vector.tensor_mul(h_new[:, cs], o_b, tau_c[:, cs])
                        if gam_next is not None:
                            nc.vector.tensor_mul(gam_next[:, cs], h_new[:, cs],
                                                 gam_next[:, cs])
                            if USE_FP8_HH:
                                nc.gpsimd.tensor_copy(
                                    hp8_next[:, col // 2, col % 2, :], gam_next[:, cs])

                    # natural-layout blend (off critical path): exact output
                    # reverse transpose on PE (frees the DMA xbar)
                    xct = psB.tile([128, NB * FD], BF16, tag="xc", name=f"xct{nt}")
                    for k in range(NB):
                        nc.tensor.matmul(xct[:, FD * k:FD * (k + 1)],
                                         xcb[:, 128 * k:128 * (k + 1)], t_idn,
                                         is_transpose=True, start=True, stop=True)
                    xcnf = npo.tile([128, NB, FD], F32, tag="xcnf", name=f"xcnf{nt}")
                    nc.vector.tensor_copy(
                        xcnf, xct[:, :].rearrange("p (k f) -> p k f", k=NB))
                    dif = npo.tile([128, NB, FD], F32, tag="dif", name=f"dif{nt}")
                    nc.gpsimd.tensor_sub(dif, vf[:, :, tl, :], xcnf)
                    nc.gpsimd.tensor_mul(dif, mf[:, :, tl, :], dif)
                    cc = npo.tile([128, NB, FD], F32, tag="cc", name=f"cc{nt}")
                    nc.gpsimd.tensor_add(cc, xcnf, dif)
                    nc.vector.tensor_reduce(out=t_xnum[:, t:t + 1], in_=dif, axis=AX.XY,
                                            op=ALU.add, apply_absolute_value=True)
                    nc.sync.dma_start(out=imps_v[:, :, t, :], in_=cc)

                    h_cur, c_cur = h_new, c_new
                    hp, hp8, cur = gam_next, hp8_next, nxt

                # ---- epilogue: y = (out_W/2) @ h~ ----
                yps = psB.tile([1, Bc], F32, tag="xc", name="yps")
                for k in range(NH):
                    nc.tensor.matmul(yps, t_out[k], h_cur[:, Bc * k:Bc * (k + 1)],
                                     start=(k == 0), stop=(k == NH - 1))
                t_y = win.tile([1, Bc], F32, tag="ty", name="t_y")
                nc.vector.tensor_copy(t_y, yps)
                nc.sync.dma_start(out=o_y[:, :], in_=t_y)
                nc.sync.dma_start(out=o_xnum[:, :], in_=t_xnum)
                nc.sync.dma_start(out=o_xden[:, :], in_=t_xden)

    return nc


def t_st_pre(nc, pre, w_st):
    t_st = pre.tile([S, FD], BF16, name="t_st")
    nc.sync.dma_start(out=t_st, in_=w_st[:, :])
    return t_st


def prep_weights(inputs, Bc=512):
    """Host-side weight repack (bf16 casts, transposes, folding)."""
    bf = ml_dtypes.bfloat16
    f32 = np.float32
    NH, NG = H // 128, 4 * H // 128

    def colmaj(b, n):  # [n*128] -> [128, n] with col j = b[128j:128j+128]
        return np.ascontiguousarray(np.asarray(b, f32).reshape(n, 128).T)

    # packed-H permutation: position q=128*j+p (fat block j, partition p)
    # holds natural H row perm[q] = 256*(j//2) + 2*p + (j%2)
    if USE_FP8_HH:
        perm = np.array([256 * (j // 2) + 2 * p + (j % 2)
                         for j in range(4) for p in range(128)])
    else:
        perm = np.arange(H)
    gperm = np.concatenate([X * H + perm for X in range(4)])

    w = {}
    w["w_fr"] = np.ascontiguousarray(np.asarray(inputs["fr_W"], f32).T).astype(bf)
    w["w_d1"] = np.ascontiguousarray(np.asarray(inputs["d1_W"], f32).T).astype(bf)
    w["w_d2"] = np.ascontiguousarray(np.asarray(inputs["d2_W"], f32).T).astype(bf)
    w["w_d3"] = np.ascontiguousarray(2.0 * np.asarray(inputs["d3_W"], f32).T[:, perm]).astype(bf)
    td_b = np.asarray(inputs["td_b"], f32)
    w["w_td"] = np.ascontiguousarray(np.concatenate(
        [np.asarray(inputs["td_W"], f32).T,
         td_b.reshape(1, H)], axis=0)[:, perm]).astype(bf)
    w["w_hist"] = np.ascontiguousarray(
        0.5 * np.asarray(inputs["hist_W"], f32).T[perm, :]).astype(bf)
    w["w_st"] = np.ascontiguousarray(np.asarray(inputs["st_W"], f32).T).astype(bf)
    w["w_real"] = np.ascontiguousarray(np.asarray(inputs["real_W"], f32).T).astype(bf)
    wih = np.asarray(inputs["lstm_Wih"], f32)
    # per-gate scale: 0.5 for i,f,o (sigmoid-as-tanh), 1.0 for g (torch order i,f,g,o)
    gsc = np.concatenate([np.full(H, 0.5, f32), np.full(H, 0.5, f32),
                          np.full(H, 1.0, f32), np.full(H, 0.5, f32)])
    lb = (np.asarray(inputs["lstm_bih"], f32) + np.asarray(inputs["lstm_bhh"], f32))
    gk = FP8_K if USE_FP8_HH else 1.0
    w["w_icc"] = np.ascontiguousarray(
        (gk * wih[:, :FD].T * gsc[None, :])[:, gperm]).astype(bf)
    w["w_im"] = np.ascontiguousarray(np.concatenate(
        [gk * wih[:, FD:].T * gsc[None, :], (gk * lb * gsc).reshape(1, 4 * H)],
        axis=0)[:, gperm]).astype(bf)
    whh_t = gk * 0.5 * np.asarray(inputs["lstm_Whh"], f32).T * gsc[None, :]
    whh_t = whh_t[:, gperm]          # M-side (gate rows) permuted
    if USE_FP8_HH:
        # natural reshape IS the perm-consistent DR packing:
        # W_dr[b,p,e] = W[256b+2p+e] = W[perm[128*(2b+e)+p]]
        w["w_hh"] = np.ascontiguousarray(
            whh_t.reshape(2, 128, 2, 4 * H)).astype(ml_dtypes.float8_e4m3)
    else:
        w["w_hh"] = np.ascontiguousarray(whh_t).astype(bf)
    w["w_out"] = np.ascontiguousarray(
        0.5 * np.asarray(inputs["out_W"], f32).T[perm, :]).astype(bf)

    w["b_fr"] = np.asarray(inputs["fr_b"], f32).reshape(S, 1)
    w["b_d1"] = colmaj(inputs["d1_b"], NH)
    w["b_d2"] = colmaj(inputs["d2_b"], NH)
    w["b_d3"] = colmaj(2.0 * np.asarray(inputs["d3_b"], f32)[perm], NH)
    w["b_stc"] = (np.asarray(inputs["st_b"], f32) + np.asarray(inputs["real_b"], f32)
                  + np.asarray(inputs["hist_b"], f32)).reshape(FD, 1)
    w["ident"] = np.eye(FD, dtype=f32).astype(bf)
    return w


_NC_CACHE = {}


def kernel(**inputs):
    from concourse.bass_utils import run_bass_kernel_spmd

    Bc = B_FULL // N_CORES
    key = (Bc, T_FULL)
    if key not in _NC_CACHE:
        nc_ = build_module(Bc=Bc, T=T_FULL, Tc=4)
        nc_.compile()
        _NC_CACHE[key] = nc_
    nc = _NC_CACHE[key]

    f32 = np.float32
    vals = np.ascontiguousarray(np.asarray(inputs["values"], f32))
    msks = np.ascontiguousarray(np.asarray(inputs["masks"], f32))
    dels = np.ascontiguousarray(np.asarray(inputs["deltas"], f32))
    stat = np.ascontiguousarray(np.asarray(inputs["statics"], f32))
    smk = np.ascontiguousarray(np.asarray(inputs["static_masks"], f32))
    labels = np.asarray(inputs["labels"], f32)

    w = prep_weights(inputs, Bc)
    in_maps = []
    for c in range(N_CORES):
        sl = slice(c * Bc, (c + 1) * Bc)
        m = dict(values=vals[sl], masks=msks[sl], deltas=dels[sl],
                 statics=stat[sl], smasks=smk[sl])
        m.update(w)
        in_maps.append(m)

    res = run_bass_kernel_spmd(nc, in_maps, core_ids=list(range(N_CORES)))
    outs = res.results
    return finish_outputs(inputs, outs)


def finish_outputs(inputs, outs):
    f32 = np.float32
    labels = np.asarray(inputs["labels"], f32)
    imputations = np.concatenate([o["imps"] for o in outs], axis=0)
    z = np.concatenate([o["ylog"][0] for o in outs]) + f32(np.asarray(inputs["out_b"], f32)[0])
    xnum = np.stack([o["xnum"] for o in outs])  # [cores, 128, T]
    xden = np.stack([o["xden"] for o in outs])

    num_t = xnum.sum(axis=(0, 1), dtype=np.float64)
    den_t = xden.sum(axis=(0, 1), dtype=np.float64)
    x_loss = float((num_t / (den_t + 1e-5)).sum())

    z = z.astype(f32)
    y_loss = float(np.mean(np.maximum(z, 0.0) - z * labels + np.log1p(np.exp(-np.abs(z)))))

    real_W = np.asarray(inputs["real_W"], f32)
    hist_W = np.asarray(inputs["hist_W"], f32)
    st_W = np.asarray(inputs["st_W"], f32)
    fr_W = np.asarray(inputs["fr_W"], f32)
    r_loss = L1 * (np.abs(real_W).sum() + np.abs(hist_W).sum() + np.abs(st_W).sum()) \
        + L2 * np.abs(np.diagonal(real_W)).sum()
    s_loss = IMPUTE_WEIGHT * L2 * np.abs(np.diagonal(fr_W)).sum()

    loss = np.float32(y_loss + float(r_loss) + IMPUTE_WEIGHT * x_loss + float(s_loss))
    y_prob = (1.0 / (1.0 + np.exp(-z))).astype(f32).reshape(-1, 1)
    return (loss, y_prob, imputations.astype(f32))


# revision 23
# speedup vs baseline: 1.1525x; 1.1525x over previous
"""Trainium2 Bass kernel for the multimodal GRU-D-style LSTM imputation model.

Self-contained: kernel(**inputs) takes the FULL inputs (B=4096) and returns
(loss, sigmoid(y_h), imputations) exactly like the reference.

Strategy: pure data-parallel over 8 NeuronCores (B=512 per core). Each core
runs the full T=128-step scan on its batch shard with bf16 matmuls
(fp32 PSUM accumulation), one activation-table set (exp/tanh/relu) by
expressing sigmoid via tanh, and fp32 state for the LSTM cell c.
Scalar losses are finished on the host from per-core partial sums.

Layout notes (per core, B=512 -> 4 partition chunks of 128):
  - transposed activations live as [feat, batch] tiles: h~=2h bf16 [128,2048]
    fat tiles (col block k = H rows 128k..128k+127), c~=2c fp32 [128,2048].
  - per-step inputs x/m/d loaded natural [128(b), k, t, 96], converted to
    bf16 (f padded to 128 with zeros) and transposed via the DMA xbar.
  - imputations c_c computed in natural layout fp32 (exact where mask=1) and
    DMA'd straight out; the transposed bf16 copy feeds the LSTM gates.
  - sigmoid(z) = (1+tanh(z/2))/2 folded into pre-scaled weights: the kernel
    carries h~=2h and c~=2c; hist_W/Whh/out_W are pre-halved, d3 pre-doubled.
"""

import numpy as np
import ml_dtypes

import concourse.bass as bass
import concourse.bacc as bacc
import concourse.tile as tile
from concourse import mybir

F32 = mybir.dt.float32
BF16 = mybir.dt.bfloat16
FP8 = mybir.dt.float8e4
USE_FP8_HH = True
FP8_K = 16.0
FWD_VIA_PE = False
AF = mybir.ActivationFunctionType
ALU = mybir.AluOpType
AX = mybir.AxisListType

B_FULL, T_FULL, FD, S, H = 4096, 128, 96, 128, 512
N_CORES = 8
IMPUTE_WEIGHT, L1, L2 = 0.3, 0.01, 0.1


def build_module(Bc=512, T=128, Tc=4, repeat=1):
    """Build the per-core Bass module. Bc = per-core batch, T = steps."""
    NB = Bc // 128          # batch chunks
    NH = H // 128           # hidden chunks (4)
    NG = 4 * H // 128       # gate M-chunks (16)
    assert Bc % 128 == 0 and T % Tc == 0

    nc = bacc.Bacc(target_bir_lowering=False)

    # ---- DRAM I/O ----
    vals = nc.declare_dram_parameter("values", [Bc, T, FD], F32, isOutput=False)
    msks = nc.declare_dram_parameter("masks", [Bc, T, FD], F32, isOutput=False)
    dels = nc.declare_dram_parameter("deltas", [Bc, T, FD], F32, isOutput=False)
    stat = nc.declare_dram_parameter("statics", [Bc, S], F32, isOutput=False)
    smk = nc.declare_dram_parameter("smasks", [Bc, S], F32, isOutput=False)

    w_fr = nc.declare_dram_parameter("w_fr", [S, S], BF16, isOutput=False)        # fr_W.T
    w_d1 = nc.declare_dram_parameter("w_d1", [S, H], BF16, isOutput=False)        # d1_W.T
    w_d2 = nc.declare_dram_parameter("w_d2", [H, H], BF16, isOutput=False)        # d2_W.T
    w_d3 = nc.declare_dram_parameter("w_d3", [H, H], BF16, isOutput=False)        # (2*d3_W).T
    w_td = nc.declare_dram_parameter("w_td", [FD + 1, H], BF16, isOutput=False)   # [td_W.T; td_b]
    w_hist = nc.declare_dram_parameter("w_hist", [H, FD], BF16, isOutput=False)   # (hist_W/2).T
    w_st = nc.declare_dram_parameter("w_st", [S, FD], BF16, isOutput=False)       # st_W.T
    w_real = nc.declare_dram_parameter("w_real", [FD, FD], BF16, isOutput=False)  # real_W.T
    w_icc = nc.declare_dram_parameter("w_icc", [FD, 4 * H], BF16, isOutput=False)  # Wih[:, :96].T
    w_im = nc.declare_dram_parameter("w_im", [FD + 1, 4 * H], BF16, isOutput=False)  # [Wih_m.T; b], ifo cols halved
    if USE_FP8_HH:
        w_hh = nc.declare_dram_parameter("w_hh", [2, 128, 2, 4 * H], FP8, isOutput=False)
    else:
        w_hh = nc.declare_dram_parameter("w_hh", [H, 4 * H], BF16, isOutput=False)  # (Whh/2).T
    w_out = nc.declare_dram_parameter("w_out", [H, 1], BF16, isOutput=False)       # (out_W/2).T

    b_fr = nc.declare_dram_parameter("b_fr", [S, 1], F32, isOutput=False)
    b_d1 = nc.declare_dram_parameter("b_d1", [128, NH], F32, isOutput=False)
    b_d2 = nc.declare_dram_parameter("b_d2", [128, NH], F32, isOutput=False)
    b_d3 = nc.declare_dram_parameter("b_d3", [128, NH], F32, isOutput=False)      # 2*d3_b
    b_stc = nc.declare_dram_parameter("b_stc", [FD, 1], F32, isOutput=False)      # st_b+real_b+hist_b
    ident = nc.declare_dram_parameter("ident", [128, 128], BF16, isOutput=False)

    o_imps = nc.declare_dram_parameter("imps", [Bc, T, FD], F32, isOutput=True)
    o_y = nc.declare_dram_parameter("ylog", [1, Bc], F32, isOutput=True)
    o_xnum = nc.declare_dram_parameter("xnum", [128, T], F32, isOutput=True)
    o_xden = nc.declare_dram_parameter("xden", [128, T], F32, isOutput=True)

    vals_v = vals.ap().rearrange("(k p) t f -> p k t f", k=NB)
    msks_v = msks.ap().rearrange("(k p) t f -> p k t f", k=NB)
    dels_v = dels.ap().rearrange("(k p) t f -> p k t f", k=NB)
    imps_v = o_imps.ap().rearrange("(k p) t f -> p k t f", k=NB)
    stat_v = stat.ap().rearrange("(k p) s -> p k s", k=NB)
    smk_v = smk.ap().rearrange("(k p) s -> p k s", k=NB)

    FB = NH * Bc  # fat tile free size (2048 at Bc=512)

    with tile.TileContext(nc) as tc:
        with tc.tile_pool(name="wp", bufs=1) as wp, \
             tc.tile_pool(name="sp1", bufs=1) as sp1, \
             tc.tile_pool(name="sp2", bufs=2) as sp2, \
             tc.tile_pool(name="psG", bufs=(2 if FWD_VIA_PE else 3), space="PSUM") as psG, \
             tc.tile_pool(name="psB", bufs=1, space="PSUM") as psB, \
             tc.tile_pool(name="psF", bufs=2, space="PSUM") as psF, \
             tc.tile_pool(name="psT", bufs=1, space="PSUM") as psT:

            # ---- persistent loop weights ----
            t_td = wp.tile([FD + 1, H], BF16, name="t_td")
            nc.sync.dma_start(out=t_td, in_=w_td[:, :])
            t_hist = [wp.tile([128, FD], BF16, tag=f"hi_{k}", name=f"hi_{k}") for k in range(NH)]
            for k in range(NH):
                nc.sync.dma_start(out=t_hist[k], in_=w_hist[128 * k:128 * (k + 1), :])
            t_real = wp.tile([FD, FD], BF16, name="t_real")
            nc.sync.dma_start(out=t_real, in_=w_real[:, :])
            t_icc = wp.tile([FD, 4 * H], BF16, name="t_icc")
            nc.sync.dma_start(out=t_icc, in_=w_icc[:, :])
            t_im = wp.tile([FD + 1, 4 * H], BF16, name="t_im")
            nc.sync.dma_start(out=t_im, in_=w_im[:, :])
            if USE_FP8_HH:
                t_hh = [wp.tile([128, 2, 4 * H], FP8, tag=f"hh_{k}", name=f"hh_{k}")
                        for k in range(2)]
                for k in range(2):
                    nc.sync.dma_start(out=t_hh[k], in_=w_hh[k, :, :, :])
            else:
                t_hh = [wp.tile([128, 4 * H], BF16, tag=f"hh_{k}", name=f"hh_{k}")
                        for k in range(NH)]
                for k in range(NH):
                    nc.sync.dma_start(out=t_hh[k], in_=w_hh[128 * k:128 * (k + 1), :])
            t_out = [wp.tile([128, 1], BF16, tag=f"ow_{k}", name=f"ow_{k}") for k in range(NH)]
            for k in range(NH):
                nc.sync.dma_start(out=t_out[k], in_=w_out[128 * k:128 * (k + 1), :])
            t_stcb = wp.tile([FD, 1], F32, name="t_stcb")
            nc.sync.dma_start(out=t_stcb, in_=b_stc[:, :])
            t_stc = wp.tile([FD, Bc], F32, name="t_stc")
            t_idn = wp.tile([128, 128], BF16, name="t_idn")
            nc.sync.dma_start(out=t_idn, in_=ident[:, :])

            # persistent accumulators / padded bf16 window tiles
            t_xnum = wp.tile([128, T], F32, name="t_xnum")
            t_xden = wp.tile([128, T], F32, name="t_xden")
            vb = [wp.tile([128, NB, Tc, 128], BF16, tag=f"vb{i}", name=f"vb{i}") for i in range(2)]
            mb = [wp.tile([128, NB, Tc, 128], BF16, tag=f"mb{i}", name=f"mb{i}") for i in range(2)]
            db = [wp.tile([128, NB, Tc, 128], BF16, tag=f"db{i}", name=f"db{i}") for i in range(2)]
            for tl_ in vb + mb + db:
                nc.gpsimd.memset(tl_, 0.0)
            for tl_ in mb + db:
                nc.gpsimd.memset(tl_[:, :, :, FD:FD + 1], 1.0)

            # ---- prologue: statics -> s_cT, h0, st_term (pool freed after) ----
            with tc.tile_pool(name="pre", bufs=1) as pre:
                t_fr = pre.tile([S, S], BF16, name="t_fr")
                nc.sync.dma_start(out=t_fr, in_=w_fr[:, :])
                t_d1 = pre.tile([S, H], BF16, name="t_d1")
                nc.sync.dma_start(out=t_d1, in_=w_d1[:, :])
                t_d2 = [pre.tile([128, H], BF16, tag=f"d2_{k}", name=f"d2_{k}") for k in range(NH)]
                t_d3 = [pre.tile([128, H], BF16, tag=f"d3_{k}", name=f"d3_{k}") for k in range(NH)]
                for k in range(NH):
                    nc.sync.dma_start(out=t_d2[k], in_=w_d2[128 * k:128 * (k + 1), :])
                    nc.sync.dma_start(out=t_d3[k], in_=w_d3[128 * k:128 * (k + 1), :])
                t_frb = pre.tile([S, 1], F32, name="t_frb")
                nc.sync.dma_start(out=t_frb, in_=b_fr[:, :])
                t_d1b = pre.tile([128, NH], F32, name="t_d1b")
                nc.sync.dma_start(out=t_d1b, in_=b_d1[:, :])
                t_d2b = pre.tile([128, NH], F32, name="t_d2b")
                nc.sync.dma_start(out=t_d2b, in_=b_d2[:, :])
                t_d3b = pre.tile([128, NH], F32, name="t_d3b")
                nc.sync.dma_start(out=t_d3b, in_=b_d3[:, :])

                st_nat = pre.tile([128, NB, S], F32, name="st_nat")
                nc.sync.dma_start(out=st_nat, in_=stat_v[:, :, :])
                sm_nat = pre.tile([128, NB, S], F32, name="sm_nat")
                nc.sync.dma_start(out=sm_nat, in_=smk_v[:, :, :])
                st_b16 = pre.tile([128, NB, S], BF16, name="st_b16")
                nc.gpsimd.tensor_copy(st_b16, st_nat)
                sm_b16 = pre.tile([128, NB, S], BF16, name="sm_b16")
                nc.gpsimd.tensor_copy(sm_b16, sm_nat)
                stT = pre.tile([S, Bc], BF16, name="stT")
                smT = pre.tile([S, Bc], BF16, name="smT")
                for k in range(NB):
                    nc.sync.dma_start_transpose(stT[:, 128 * k:128 * (k + 1)], st_b16[:, k, :])
                    nc.sync.dma_start_transpose(smT[:, 128 * k:128 * (k + 1)], sm_b16[:, k, :])
                sh_ps = psG.tile([S, Bc], F32, tag="g", name="sh_ps")
                nc.tensor.matmul(sh_ps, t_fr, stT, start=True, stop=True)
                sh = pre.tile([S, Bc], BF16, name="sh")
                nc.scalar.activation(out=sh, in_=sh_ps, func=AF.Identity, bias=t_frb, scale=1.0)
                sdif = pre.tile([S, Bc], BF16, name="sdif")
                nc.vector.tensor_sub(sdif, stT, sh)
                nc.vector.tensor_mul(sdif, smT, sdif)
                scT = pre.tile([S, Bc], BF16, name="scT")
                nc.vector.tensor_add(scT, sh, sdif)

                # dense chain h1 -> h2 -> h3 (h3 doubled via doubled d3 weights)
                h1 = pre.tile([128, FB], BF16, name="h1")
                for j in range(NH):
                    ps = psG.tile([128, Bc], F32, tag="g", name=f"h1ps{j}")
                    nc.tensor.matmul(ps, t_d1[:, 128 * j:128 * (j + 1)], scT,
                                     start=True, stop=True)
                    nc.scalar.activation(out=h1[:, Bc * j:Bc * (j + 1)], in_=ps, func=AF.Relu,
                                         bias=t_d1b[:, j:j + 1], scale=1.0)
                h2 = pre.tile([128, FB], BF16, name="h2")
                for j in range(NH):
                    ps = psG.tile([128, Bc], F32, tag="g", name=f"h2ps{j}")
                    for k in range(NH):
                        nc.tensor.matmul(ps, t_d2[k][:, 128 * j:128 * (j + 1)],
                                         h1[:, Bc * k:Bc * (k + 1)],
                                         start=(k == 0), stop=(k == NH - 1))
                    nc.scalar.activation(out=h2[:, Bc * j:Bc * (j + 1)], in_=ps, func=AF.Relu,
                                         bias=t_d2b[:, j:j + 1], scale=1.0)
                h_cur = sp2.tile([128, FB], BF16, tag="h", name="h0")
                for j in range(NH):
                    ps = psG.tile([128, Bc], F32, tag="g", name=f"h3ps{j}")
                    for k in range(NH):
                        nc.tensor.matmul(ps, t_d3[k][:, 128 * j:128 * (j + 1)],
                                         h2[:, Bc * k:Bc * (k + 1)],
                                         start=(k == 0), stop=(k == NH - 1))
                    nc.scalar.activation(out=h_cur[:, Bc * j:Bc * (j + 1)], in_=ps, func=AF.Relu,
                                         bias=t_d3b[:, j:j + 1], scale=1.0)

                stc_ps = psB.tile([FD, Bc], F32, tag="xc", name="stc_ps")
                nc.tensor.matmul(stc_ps, t_st_pre(nc, pre, w_st), scT, start=True, stop=True)
                nc.scalar.activation(out=t_stc, in_=stc_ps, func=AF.Identity,
                                     bias=t_stcb, scale=1.0)

            c_cur = sp2.tile([128, FB], BF16, tag="c", name="c0")
            nc.vector.memset(c_cur, 0.0)

            with tc.tile_pool(name="win", bufs=2) as win, \
                 tc.tile_pool(name="tp", bufs=2) as tp, \
                 tc.tile_pool(name="xp", bufs=2) as xp, \
                 tc.tile_pool(name="np_", bufs=2) as npo:

                wins = {}

                def load_window(w, lbl=None):
                    lbl = w if lbl is None else f"{w}_{lbl}"
                    t0 = w * Tc
                    wpi = w % 2
                    vf = win.tile([128, NB, Tc, FD], F32, tag="vf", name=f"vf{lbl}")
                    nc.sync.dma_start(out=vf, in_=vals_v[:, :, t0:t0 + Tc, :])
                    mf = win.tile([128, NB, Tc, FD], F32, tag="mf", name=f"mf{lbl}")
                    nc.sync.dma_start(out=mf, in_=msks_v[:, :, t0:t0 + Tc, :])
                    df = win.tile([128, NB, Tc, FD], F32, tag="df", name=f"df{lbl}")
                    nc.sync.dma_start(out=df, in_=dels_v[:, :, t0:t0 + Tc, :])
                    nc.gpsimd.tensor_copy(vb[wpi][:, :, :, 0:FD], vf)
                    nc.gpsimd.tensor_copy(mb[wpi][:, :, :, 0:FD], mf)
                    nc.gpsimd.tensor_copy(db[wpi][:, :, :, 0:FD], df)
                    nc.vector.tensor_reduce(
                        out=t_xden[:, t0:t0 + Tc], in_=mf.transpose([0, 2, 1, 3]),
                        axis=AX.XY, op=ALU.add)
                    wins[w] = (vf, mf, df)

                def transposes(t, lbl=None):
                    lbl = t if lbl is None else f"{t}_{lbl}"
                    w, tl = t // Tc, t % Tc
                    wpi = w % 2
                    vT = tp.tile([128, Bc], BF16, tag="vT", name=f"vT{lbl}")
                    mT = tp.tile([128, Bc], BF16, tag="mT", name=f"mT{lbl}")
                    dT = tp.tile([128, Bc], BF16, tag="dT", name=f"dT{lbl}")
                    if FWD_VIA_PE:
                        for src_t, dst, eng in ((vb, vT, "v"), (mb, mT, "a"), (db, dT, "a")):
                            pf = psF.tile([128, Bc], BF16, tag="tpp", name=f"tpp{lbl}_{eng}{id(dst)%97}")
                            for k in range(NB):
                                nc.tensor.matmul(pf[:, 128 * k:128 * (k + 1)],
                                                 src_t[wpi][:, k, tl, :], t_idn,
                                                 is_transpose=True, start=True, stop=True)
                            if eng == "v":
                                nc.vector.tensor_copy(dst, pf)
                            else:
                                nc.scalar.copy(dst, pf)
                    else:
                        for k in range(NB):
                            nc.sync.dma_start_transpose(vT[:, 128 * k:128 * (k + 1)],
                                                        vb[wpi][:, k, tl, :])
                            nc.sync.dma_start_transpose(mT[:, 128 * k:128 * (k + 1)],
                                                        mb[wpi][:, k, tl, :])
                            nc.sync.dma_start_transpose(dT[:, 128 * k:128 * (k + 1)],
                                                        db[wpi][:, k, tl, :])
                    return vT, mT, dT

                def gamma_of(t, dT):
                    # gamma = min(exp(-(z + td_b)), 1): bias via ones-row K=97
                    gam = sp2.tile([128, FB], BF16, tag="gam", name=f"gam{t}")
                    for j2 in range(NH // 2):
                        ps = psG.tile([128, 2 * Bc], F32, tag="g", name=f"gps{t}_{j2}")
                        for q in range(2):
                            j = 2 * j2 + q
                            nc.tensor.matmul(ps[:, Bc * q:Bc * (q + 1)],
                                             t_td[:, 128 * j:128 * (j + 1)],
                                             dT[0:FD + 1, :], start=True, stop=True)
                        nc.scalar.activation(out=gam[:, 2 * Bc * j2:2 * Bc * (j2 + 1)],
                                             in_=ps, func=AF.Exp, scale=-1.0)
                    nc.vector.tensor_scalar_min(gam, gam, 1.0)
                    return gam

                def make_hp8(t):
                    return sp2.tile([128, 2, 2, Bc], FP8, tag="hp8", name=f"hp8{t}")

                # ---- startup: window 0, transposes/gamma/hp for t=0 ----
                load_window(0)
                cur = hp = hp8 = None
                for nt in range(repeat * T):
                    t = nt % T
                    if t == 0:  # (re)start a pass (repeat>1 is a timing rig)
                        cur = transposes(0) if nt == 0 else transposes(0, nt)
                        hp = gamma_of(nt, cur[2])
                        nc.vector.tensor_mul(hp, h_cur, hp)
                        hp8 = None
                        if USE_FP8_HH:
                            hp8 = make_hp8(nt)
                            nc.gpsimd.tensor_copy(
                                hp8, hp[:, :].rearrange("p (b e n) -> p b e n", b=2, e=2))
                    w, tl = t // Tc, t % Tc
                    if tl == 0 and w + 1 < T // Tc:
                        load_window(w + 1, nt)
                    vf, mf, df = wins[w]
                    vT, mT, dT = cur
                    if t + 1 < T:
                        nxt = transposes(t + 1, nt + 1)
                        gam_next = gamma_of(nt + 1, nxt[2])
                        hp8_next = make_hp8(nt + 1) if USE_FP8_HH else None
                    else:
                        nxt = gam_next = hp8_next = None

                    # x_cT = real_W @ xT + (hist_W/2) @ hp + st_term(+biases)
                    xps = psB.tile([FD, Bc], F32, tag="xc", name=f"xc{nt}")
                    nc.tensor.matmul(xps, t_real, vT[0:FD, :], start=True, stop=False)
                    for k in range(NH):
                        nc.tensor.matmul(xps, t_hist[k], hp[:, Bc * k:Bc * (k + 1)],
                                         start=False, stop=(k == NH - 1))
                    xcb = xp.tile([FD, Bc], BF16, tag="xcb", name=f"xcb{nt}")
                    nc.vector.tensor_add(xcb, xps, t_stc)

                    # transposed blend for the gate input c_cT
                    difT = xp.tile([FD, Bc], BF16, tag="difT", name=f"difT{nt}")
                    nc.vector.tensor_sub(difT, vT[0:FD, :], xcb)
                    nc.vector.tensor_mul(difT, mT[0:FD, :], difT)
                    ccT = xp.tile([FD, Bc], BF16, tag="ccT", name=f"ccT{nt}")
                    nc.vector.tensor_add(ccT, xcb, difT)

                    # LSTM gates, biases+0.5-scales folded into the GEMM:
                    # per 128-row chunk the blocks (g,i)/(f,o) share one
                    # 2-bank psum + one ACT; the cell/hidden tail and the
                    # NEXT step's decayed hidden pipeline per chunk.
                    c_new = sp2.tile([128, FB], BF16, tag="c", name=f"c{nt}")
                    tau_c = sp1.tile([128, FB], BF16, tag="tc", name=f"tc{nt}")
                    h_new = sp2.tile([128, FB], BF16, tag="h", name=f"h{nt}")
                    for col in range(NH):
                        ga = sp2.tile([128, 4, Bc], BF16, tag="ga", name=f"ga{nt}_{col}")
                        for pair in ((2, 0), (1, 3)):  # (g,i) then (f,o)
                            ps = psG.tile([128, 2 * Bc], F32, tag="g",
                                          name=f"g{nt}_{col}_{pair[0]}")
                            for q, gate in enumerate(pair):
                                j = gate * NH + col
                                pso = ps[:, Bc * q:Bc * (q + 1)]
                                nc.tensor.matmul(pso, t_im[:, 128 * j:128 * (j + 1)],
                                                 mT[0:FD + 1, :], start=True, stop=False)
                                if USE_FP8_HH:
                                    for k in range(2):
                                        nc.tensor.matmul(
                                            pso, t_hh[k][:, :, 128 * j:128 * (j + 1)],
                                            hp8[:, k, :, :], start=False, stop=False,
                                            perf_mode=mybir.MatmulPerfMode.DoubleRow)
                                else:
                                    for k in range(NH):
                                        nc.tensor.matmul(
                                            pso, t_hh[k][:, 128 * j:128 * (j + 1)],
                                            hp[:, Bc * k:Bc * (k + 1)],
                                            start=False, stop=False)
                                nc.tensor.matmul(pso, t_icc[:, 128 * j:128 * (j + 1)], ccT,
                                                 start=False, stop=True)
                            blk = 0 if pair[0] == 2 else 2
                            nc.scalar.activation(out=ga[:, blk:blk + 2, :], in_=ps,
                                                 func=AF.Tanh,
                                                 scale=(1.0 / FP8_K) if USE_FP8_HH else 1.0)
                        g_b, i_b, f_b, o_b = ga[:, 0, :], ga[:, 1, :], ga[:, 2, :], ga[:, 3, :]
                        cs = slice(Bc * col, Bc * (col + 1))
                        # c~_new = (0.5+0.5*tau_f)*c~ + (1+tau_i)*g   (chunk col)
                        nc.vector.tensor_scalar(f_b, f_b, 0.5, 0.5, ALU.mult, ALU.add)
                        nc.vector.tensor_scalar(i_b, i_b, 1.0, None, ALU.add)
                        nc.vector.tensor_mul(g_b, i_b, g_b)
                        nc.vector.tensor_mul(c_cur[:, cs], f_b, c_cur[:, cs])
                        nc.vector.tensor_add(c_new[:, cs], c_cur[:, cs], g_b)
                        # h~_new = (1+tau_o)*tau_c,  tau_c = tanh(c~/2)
                        nc.scalar.activation(out=tau_c[:, cs], in_=c_new[:, cs],
                                             func=AF.Tanh, scale=0.5)
                        nc.vector.tensor_scalar(o_b, o_b, 1.0, None, ALU.add)
                        nc.vector.tensor_mul(h_new[:, cs], o_b, tau_c[:, cs])
                        if gam_next is not None:
                            nc.vector.tensor_mul(gam_next[:, cs], h_new[:, cs],
                                                 gam_next[:, cs])
                            if USE_FP8_HH:
                                nc.gpsimd.tensor_copy(
                                    hp8_next[:, col // 2, col % 2, :], gam_next[:, cs])

                    # natural-layout blend (off critical path): exact output
                    # reverse transpose on PE (frees the DMA xbar)
                    xct = psT.tile([128, NB * FD], BF16, tag="xct", name=f"xct{nt}")
                    for k in range(NB):
                        nc.tensor.matmul(xct[:, FD * k:FD * (k + 1)],
                                         xcb[:, 128 * k:128 * (k + 1)], t_idn[0:FD, 0:FD],
                                         is_transpose=True, start=True, stop=True)
                    xcnf = npo.tile([128, NB, FD], F32, tag="xcnf", name=f"xcnf{nt}")
                    nc.vector.tensor_copy(
                        xcnf, xct[:, :].rearrange("p (k f) -> p k f", k=NB))
                    dif = npo.tile([128, NB, FD], F32, tag="dif", name=f"dif{nt}")
                    nc.gpsimd.tensor_sub(dif, vf[:, :, tl, :], xcnf)
                    nc.gpsimd.tensor_mul(dif, mf[:, :, tl, :], dif)
                    cc = npo.tile([128, NB, FD], F32, tag="cc", name=f"cc{nt}")
                    nc.gpsimd.tensor_add(cc, xcnf, dif)
                    nc.vector.tensor_reduce(out=t_xnum[:, t:t + 1], in_=dif, axis=AX.XY,
                                            op=ALU.add, apply_absolute_value=True)
                    nc.sync.dma_start(out=imps_v[:, :, t, :], in_=cc)

                    h_cur, c_cur = h_new, c_new
                    hp, hp8, cur = gam_next, hp8_next, nxt

                # ---- epilogue: y = (out_W/2) @ h~ ----
                yps = psB.tile([1, Bc], F32, tag="xc", name="yps")
                for k in range(NH):
                    nc.tensor.matmul(yps, t_out[k], h_cur[:, Bc * k:Bc * (k + 1)],
                                     start=(k == 0), stop=(k == NH - 1))
                t_y = win.tile([1, Bc], F32, tag="ty", name="t_y")
                nc.vector.tensor_copy(t_y, yps)
                nc.sync.dma_start(out=o_y[:, :], in_=t_y)
                nc.sync.dma_start(out=o_xnum[:, :], in_=t_xnum)
                nc.sync.dma_start(out=o_xden[:, :], in_=t_xden)

    return nc


def t_st_pre(nc, pre, w_st):
    t_st = pre.tile([S, FD], BF16, name="t_st")
    nc.sync.dma_start(out=t_st, in_=w_st[:, :])
    return t_st


def prep_weights(inputs, Bc=512):
    """Host-side weight repack (bf16 casts, transposes, folding)."""
    bf = ml_dtypes.bfloat16
    f32 = np.float32
    NH, NG = H // 128, 4 * H // 128

    def colmaj(b, n):  # [n*128] -> [128, n] with col j = b[128j:128j+128]
        return np.ascontiguousarray(np.asarray(b, f32).reshape(n, 128).T)

    # packed-H permutation: position q=128*j+p (fat block j, partition p)
    # holds natural H row perm[q] = 256*(j//2) + 2*p + (j%2)
    if USE_FP8_HH:
        perm = np.array([256 * (j // 2) + 2 * p + (j % 2)
                         for j in range(4) for p in range(128)])
    else:
        perm = np.arange(H)
    gperm = np.concatenate([X * H + perm for X in range(4)])

    w = {}
    w["w_fr"] = np.ascontiguousarray(np.asarray(inputs["fr_W"], f32).T).astype(bf)
    w["w_d1"] = np.ascontiguousarray(np.asarray(inputs["d1_W"], f32).T).astype(bf)
    w["w_d2"] = np.ascontiguousarray(np.asarray(inputs["d2_W"], f32).T).astype(bf)
    w["w_d3"] = np.ascontiguousarray(2.0 * np.asarray(inputs["d3_W"], f32).T[:, perm]).astype(bf)
    td_b = np.asarray(inputs["td_b"], f32)
    w["w_td"] = np.ascontiguousarray(np.concatenate(
        [np.asarray(inputs["td_W"], f32).T,
         td_b.reshape(1, H)], axis=0)[:, perm]).astype(bf)
    w["w_hist"] = np.ascontiguousarray(
        0.5 * np.asarray(inputs["hist_W"], f32).T[perm, :]).astype(bf)
    w["w_st"] = np.ascontiguousarray(np.asarray(inputs["st_W"], f32).T).astype(bf)
    w["w_real"] = np.ascontiguousarray(np.asarray(inputs["real_W"], f32).T).astype(bf)
    wih = np.asarray(inputs["lstm_Wih"], f32)
    # per-gate scale: 0.5 for i,f,o (sigmoid-as-tanh), 1.0 for g (torch order i,f,g,o)
    gsc = np.concatenate([np.full(H, 0.5, f32), np.full(H, 0.5, f32),
                          np.full(H, 1.0, f32), np.full(H, 0.5, f32)])
    lb = (np.asarray(inputs["lstm_bih"], f32) + np.asarray(inputs["lstm_bhh"], f32))
    gk = FP8_K if USE_FP8_HH else 1.0
    w["w_icc"] = np.ascontiguousarray(
        (gk * wih[:, :FD].T * gsc[None, :])[:, gperm]).astype(bf)
    w["w_im"] = np.ascontiguousarray(np.concatenate(
        [gk * wih[:, FD:].T * gsc[None, :], (gk * lb * gsc).reshape(1, 4 * H)],
        axis=0)[:, gperm]).astype(bf)
    whh_t = gk * 0.5 * np.asarray(inputs["lstm_Whh"], f32).T * gsc[None, :]
    whh_t = whh_t[:, gperm]          # M-side (gate rows) permuted
    if USE_FP8_HH:
        # natural reshape IS the perm-consistent DR packing:
        # W_dr[b,p,e] = W[256b+2p+e] = W[perm[128*(2b+e)+p]]
        w["w_hh"] = np.ascontiguousarray(
            whh_t.reshape(2, 128, 2, 4 * H)).astype(ml_dtypes.float8_e4m3)
    else:
        w["w_hh"] = np.ascontiguousarray(whh_t).astype(bf)
    w["w_out"] = np.ascontiguousarray(
        0.5 * np.asarray(inputs["out_W"], f32).T[perm, :]).astype(bf)

    w["b_fr"] = np.asarray(inputs["fr_b"], f32).reshape(S, 1)
    w["b_d1"] = colmaj(inputs["d1_b"], NH)
    w["b_d2"] = colmaj(inputs["d2_b"], NH)
    w["b_d3"] = colmaj(2.0 * np.asarray(inputs["d3_b"], f32)[perm], NH)
    w["b_stc"] = (np.asarray(inputs["st_b"], f32) + np.asarray(inputs["real_b"], f32)
                  + np.asarray(inputs["hist_b"], f32)).reshape(FD, 1)
    w["ident"] = np.eye(128, dtype=f32).astype(bf)
    return w


_NC_CACHE = {}


def kernel(**inputs):
    from concourse.bass_utils import run_bass_kernel_spmd

    Bc = B_FULL // N_CORES
    key = (Bc, T_FULL)
    if key not in _NC_CACHE:
        nc_ = build_module(Bc=Bc, T=T_FULL, Tc=4)
        nc_.compile()
        _NC_CACHE[key] = nc_
    nc = _NC_CACHE[key]

    f32 = np.float32
    vals = np.ascontiguousarray(np.asarray(inputs["values"], f32))
    msks = np.ascontiguousarray(np.asarray(inputs["masks"], f32))
    dels = np.ascontiguousarray(np.asarray(inputs["deltas"], f32))
    stat = np.ascontiguousarray(np.asarray(inputs["statics"], f32))
    smk = np.ascontiguousarray(np.asarray(inputs["static_masks"], f32))
    labels = np.asarray(inputs["labels"], f32)

    w = prep_weights(inputs, Bc)
    in_maps = []
    for c in range(N_CORES):
        sl = slice(c * Bc, (c + 1) * Bc)
        m = dict(values=vals[sl], masks=msks[sl], deltas=dels[sl],
                 statics=stat[sl], smasks=smk[sl])
        m.update(w)
        in_maps.append(m)

    res = run_bass_kernel_spmd(nc, in_maps, core_ids=list(range(N_CORES)))
    outs = res.results
    return finish_outputs(inputs, outs)


def finish_outputs(inputs, outs):
    f32 = np.float32
    labels = np.asarray(inputs["labels"], f32)
    imputations = np.concatenate([o["imps"] for o in outs], axis=0)
    z = np.concatenate([o["ylog"][0] for o in outs]) + f32(np.asarray(inputs["out_b"], f32)[0])
    xnum = np.stack([o["xnum"] for o in outs])  # [cores, 128, T]
    xden = np.stack([o["xden"] for o in outs])

    num_t = xnum.sum(axis=(0, 1), dtype=np.float64)
    den_t = xden.sum(axis=(0, 1), dtype=np.float64)
    x_loss = float((num_t / (den_t + 1e-5)).sum())

    z = z.astype(f32)
    y_loss = float(np.mean(np.maximum(z, 0.0) - z * labels + np.log1p(np.exp(-np.abs(z)))))

    real_W = np.asarray(inputs["real_W"], f32)
    hist_W = np.asarray(inputs["hist_W"], f32)
    st_W = np.asarray(inputs["st_W"], f32)
    fr_W = np.asarray(inputs["fr_W"], f32)
    r_loss = L1 * (np.abs(real_W).sum() + np.abs(hist_W).sum() + np.abs(st_W).sum()) \
        + L2 * np.abs(np.diagonal(real_W)).sum()
    s_loss = IMPUTE_WEIGHT * L2 * np.abs(np.diagonal(fr_W)).sum()

    loss = np.float32(y_loss + float(r_loss) + IMPUTE_WEIGHT * x_loss + float(s_loss))
    y_prob = (1.0 / (1.0 + np.exp(-z))).astype(f32).reshape(-1, 1)
    return (loss, y_prob, imputations.astype(f32))
